# revision 25
# baseline (speedup 1.0000x reference)
# Laplacian normalization kernel for Trainium2 (8 NeuronCores, SPMD).
#
# out = d^-1/2[:, None] * A * d^-1/2[None, :],  d_i = sum_j A[i, j],  A: [8192, 8192] f32
#
# The rel-err gate (2e-2) admits bf16 storage end-to-end: the host downcasts
# A to bf16 (round-to-nearest-even), the device reads/writes bf16, the host
# upcasts the result. Emulated worst-case rel err of the exact device chain
# is 1.33e-2. bf16 halves HBM traffic in both directions vs f32 AND lets the
# entire 16MB per-core shard stay resident in SBUF between the two passes
# (no pass-2 re-read): per-core DMA is ~35MB vs the f32 baseline's ~88MB.
#
# Sharding: row-wise across 8 cores (1024 rows each). Row sums are local;
# the column-scale vector needs the full d^-1/2 [8192] via a tiny bf16
# AllGather (2KB in, 16KB out per core).
#
# Measured engine rates (v1 trace): DVE reduce [128,8192]bf16->f32 10.4us,
# DVE STT 0.73 Gelem/s/part, AllGather latency ~26us, ~13us fixed preamble.
# The schedule works around those:
#   pass 1 (load-paced, ~48us): tile halves stream on the two HWDGE rings;
#     whole-tile reduces alternate DVE (reduce_sum) / ACT (activation Copy
#     with accum_out) so neither engine falls behind the 5.9us/tile arrival
#     pace. The last tile loads in 4 x 512KB chunks with per-chunk reduces
#     so the post-last-load tail is ~2us, not a 10.4us whole-tile reduce.
#   gap (AllGather, ~26us): both compute engines are otherwise idle, so ACT
#     spends the window prescaling A *= r in place (activation Copy with
#     per-partition f32 scale keeps r unquantized; costs one extra bf16
#     rounding, net accuracy is BETTER than quantizing r to bf16).
#   pass 2 (store-paced, ~48us): DVE runs plain tensor_tensor (A*r)*c —
#     2x-perf-mode eligible (all operands bf16), unlike STT — and stores
#     stream behind it.
#
# Queue discipline: only sync and ACT have HWDGE rings (gpsimd has SWDGE),
# and every dma_start costs its issuing engine ~0.7us dispatch. ACT spends
# pass 2 computing prescales, so: loads split sync/ACT (pass 1, ACT has
# slack); the collective input and cvec broadcast go on sync (a gated DMA
# on the in-order ACT ring would head-of-line block the prescales behind
# it); stores alternate sync/gpsimd, with ACT's ring taking one late tile
# once its prescales are done. Stores are whole-tile 2MB contiguous except
# the first/last tiles (column chunks: early stream start, short tail) —
# partition-sliced half-tile stores measurably stall DVE and are avoided.

import numpy as np
import ml_dtypes

N = 8192
NCORES = 8
R = N // NCORES  # 1024 rows per core
P = 128          # SBUF partitions
T = R // P       # 8 row-tiles of [128, 8192] per core
NCH = 4          # 2048-col chunks for pass 2 / last-tile loads
W = N // NCH
HALF = N // 2

_cache = {}


def _build():
    import concourse.bacc as bacc
    import concourse.mybir as mybir
    import concourse.tile as tile
    from concourse import masks

    f32 = mybir.dt.float32
    bf16 = mybir.dt.bfloat16
    X = mybir.AxisListType.X
    mult = mybir.AluOpType.mult
    Copy = mybir.ActivationFunctionType.Copy

    nc = bacc.Bacc(
        "TRN2", target_bir_lowering=False, debug=False, num_devices=NCORES
    )
    a = nc.dram_tensor("a_shard", [R, N], bf16, kind="ExternalInput").ap()
    out = nc.dram_tensor("out_shard", [R, N], bf16, kind="ExternalOutput").ap()

    a_t = a.rearrange("(t p) n -> t p n", p=P)
    o_t = out.rearrange("(t p) n -> t p n", p=P)

    with tile.TileContext(nc) as tc:
        with (
            tc.tile_pool(name="cpool", bufs=1) as cpool,
            tc.tile_pool(name="vpool", bufs=1) as vpool,
            tc.tile_pool(name="psum", bufs=1, space="PSUM") as psum,
            tc.tile_pool(name="dram", bufs=1, space="DRAM") as dram,
        ):
            big = [
                cpool.tile([P, N], bf16, tag=f"c{t}", name=f"c{t}")
                for t in range(T)
            ]
            cvec = vpool.tile([P, N], bf16, tag="cvec")
            scr = vpool.tile([P, N], bf16, tag="scr")  # ACT-reduce dump
            hpart = vpool.tile([P, NCH], f32, tag="hpart")
            dsum = vpool.tile([P, T], f32, tag="dsum")
            dinv = vpool.tile([P, T], f32, tag="dinv")
            ident = vpool.tile([P, P], f32, tag="ident")
            tp_sq = vpool.tile([T, P], f32, tag="tp_sq")
            tp_rec = vpool.tile([T, P], f32, tag="tp_rec")
            tp_bf = vpool.tile([T, P], bf16, tag="tp_bf")
            dsum_tpp = psum.tile([T, P], f32, tag="dsum_tpp")
            dinv_tpp = psum.tile([P, T], f32, tag="dinv_tpp")
            dloc = dram.tile([1, R], bf16, tag="dloc")
            dfull = dram.tile([1, N], bf16, tag="dfull")

            masks.make_identity(nc, ident[:, :])

            def act_reduce(dst, src):
                # row-sum on the scalar engine: out=scratch is a dumped
                # side effect, accum_out carries the sum (f32)
                nc.scalar.activation(
                    out=scr[:, : src.shape[1]],
                    in_=src,
                    func=Copy,
                    accum_out=dst,
                )

            # pass 1: tiles 0..6 stream as two 1MB halves (one per ring) so
            # tile t is fully resident ~5.9us*(t+1) after stream start;
            # whole-tile reduces alternate DVE/ACT by arrival
            for t in range(T - 1):
                nc.sync.dma_start(out=big[t][:, :HALF], in_=a_t[t][:, :HALF])
                nc.scalar.dma_start(out=big[t][:, HALF:], in_=a_t[t][:, HALF:])
                if t % 2 == 0:
                    nc.vector.reduce_sum(
                        out=dsum[:, t : t + 1], in_=big[t][:, :], axis=X
                    )
                else:
                    act_reduce(dsum[:, t : t + 1], big[t][:, :])
            # last tile in 4 chunks with a narrow 512-col final chunk so
            # the post-last-load reduce tail is ~0.6us; reduces split so
            # whichever engine is free picks up the tail quickly
            t = T - 1
            ld = [nc.sync, nc.scalar]
            bnds = [0, 2560, 5120, 7680, N]
            for h in range(NCH):
                cols = slice(bnds[h], bnds[h + 1])
                ld[h % 2].dma_start(out=big[t][:, cols], in_=a_t[t][:, cols])
                if h % 2 == 0:
                    act_reduce(hpart[:, h : h + 1], big[t][:, cols])
                else:
                    nc.vector.reduce_sum(
                        out=hpart[:, h : h + 1], in_=big[t][:, cols], axis=X
                    )
            act_reduce(dsum[:, t : t + 1], hpart[:, :])

            # critical chain to the collective: transpose dsum FIRST (PE is
            # free the moment the last reduce lands), then rsqrt on the
            # [8, 128] transposed view, cast bf16, one contiguous 2KB DMA.
            # The prescale-path dinv is derived FROM this chain's tp_rec
            # (PE back-transpose + DVE copy) rather than recomputed, so the
            # list scheduler cannot hoist any prescale work above the
            # collective's critical chain (it cost ~3us in the v3 trace).
            nc.tensor.transpose(dsum_tpp[:, :], dsum[:, :], ident[:, :])
            nc.scalar.sqrt(tp_sq[:, :], dsum_tpp[:, :])
            # approx reciprocal (~51 ULP) is ~5x faster on this hop and its
            # error is 3 orders below the bf16 chain's; inputs are
            # sqrt(degree) ~ 64, far from the undefined edge cases
            nc.vector.reciprocal_approx_fast(tp_rec[:, :], tp_sq[:, :])
            nc.vector.tensor_copy(out=tp_bf[:, :], in_=tp_rec[:, :])
            nc.sync.dma_start(out=dloc[0, :], in_=tp_bf[:, :])

            # d^-1/2 back in [128, T] layout for the prescale scale operand
            # (r stays f32, never quantized)
            nc.tensor.transpose(
                dinv_tpp[:, :], tp_rec[:, :], ident[:T, :T]
            )
            nc.vector.tensor_copy(out=dinv[:, :], in_=dinv_tpp[:, :])

            nc.gpsimd.collective_compute(
                "AllGather",
                mybir.AluOpType.bypass,
                replica_groups=[list(range(NCORES))],
                ins=[dloc[0, :].opt()],
                outs=[dfull[0, :].opt()],
            )

            # replicate the gathered vector across partitions; split over
            # the sync and gpsimd rings (both idle while gated on the
            # collective) so all four chunks land concurrently and the
            # sync ring reaches its first store sooner
            bcq = [nc.sync, nc.gpsimd]
            for h in range(NCH):
                cols = slice(h * W, (h + 1) * W)
                bcq[h % 2].dma_start(
                    out=cvec[:, cols],
                    in_=dfull[0:1, cols].to_broadcast((P, W)),
                )

            # prescale A *= r on ACT: no cvec dependency, so this fills the
            # otherwise-idle collective window; tile-major order matches the
            # TT consumption order below
            for t in range(T):
                for h in range(NCH):
                    cols = slice(h * W, (h + 1) * W)
                    nc.scalar.mul(
                        big[t][:, cols], big[t][:, cols], dinv[:, t : t + 1]
                    )

            # pass 2: (A*r) * c on DVE (tensor_tensor, 2x eligible),
            # tile-major. Stores are the pass-2 bottleneck (writes sustain
            # ~317GB/s while fed, but ring-feed gaps starved the drain tail
            # to ~224GB/s), so the schedule keeps every queue pulling until
            # the end: t0 and t7 store as column chunks (early stream
            # start / short tail), middle tiles as contiguous 1MB
            # row-halves, with ACT's ring joining for the late tiles (its
            # prescales are done by then and its stores sit after them in
            # queue order)
            # ring assignment balances BYTES so all three queues drain
            # together (the trace showed gpsimd's SWDGE holding 8MB and
            # finishing ~10us after the others went idle): sync 6MB,
            # gpsimd 6MB, ACT 4MB — ACT takes the two latest-ready tiles
            # since its ring only frees up once the prescales are done.
            # Whole-tile stores only (partition-sliced half-tile stores
            # measurably stall DVE).
            st2 = [nc.sync, nc.gpsimd]
            nst = 0
            for t in range(T):
                for h in range(NCH):
                    cols = slice(h * W, (h + 1) * W)
                    tl = big[t][:, cols]
                    nc.vector.tensor_tensor(
                        out=tl, in0=tl, in1=cvec[:, cols], op=mult
                    )
                    if t == 0 or t == T - 1:
                        st2[nst % 2].dma_start(out=o_t[t][:, cols], in_=tl)
                        nst += 1
                if 0 < t < T - 1:
                    q = nc.scalar if t in (5, 6) else st2[nst % 2]
                    q.dma_start(out=o_t[t][:, :], in_=big[t][:, :])
                    nst += 1

    nc.compile()
    return nc


def kernel(adjacency_matrix, _trace=False):
    from concourse.bass_utils import run_bass_kernel_spmd

    A = np.asarray(adjacency_matrix)
    assert A.shape == (N, N), A.shape
    A_bf = A.astype(ml_dtypes.bfloat16)

    if "nc" not in _cache:
        _cache["nc"] = _build()
    nc = _cache["nc"]

    in_maps = [{"a_shard": A_bf[c * R : (c + 1) * R]} for c in range(NCORES)]
    res = run_bass_kernel_spmd(
        nc, in_maps, core_ids=list(range(NCORES)), trace=_trace
    )
    _cache["last"] = res
    return np.concatenate(
        [res.results[c]["out_shard"] for c in range(NCORES)], axis=0
    ).astype(np.float32)
